# revision 1
# baseline (speedup 1.0000x reference)
"""ContextualConv1d Trainium2 kernel (polyphase scheme).

Problem: grouped conv1d (N=32, C_in=256, L=4096, C_out=256, K=9, groups=4,
pad=4) + broadcast context term c @ c_weight.T + bias.

Sharding: data-parallel over batch N across 8 cores (4 batches/core).

Conv strategy — polyphase decomposition for full PE utilization:
  x is split host-side into even/odd phases. For one group, the matmul
  contraction packs (64 channels x 2 input phases) = 128 rows, and the
  output partitions pack (64 out channels x 2 output parities) = 128.
  The K=9 conv then becomes 5 accumulating float32r matmuls (phase shifts
  s=0..4) with dense 128x128 stationary operands (~90% MAC utilization,
  vs 50% for the naive group-pair block-diagonal form):

    ps[(o,d), m] += lhsT_s[(i,ph), (o,d)] * x2[(i,ph), m+s]
    lhsT_s[(i,0),(o,0)] = W[o,i,2s]    lhsT_s[(i,1),(o,0)] = W[o,i,2s+1]
    lhsT_s[(i,0),(o,1)] = W[o,i,2s-1]  lhsT_s[(i,1),(o,1)] = W[o,i,2s]
    (out-of-range taps are zero blocks)

  y[o, 2m+d] = ps[(o,d), m]. The phase split of x and the parity merge of
  the output are free host-side numpy reshapes (done during shard/unshard).

Precision/perf choices (all measured on HW via paired repeat-loop timing):
  - Conv inputs in fp16 (CONV_DTYPE): same PE matmul rate as float32r
    (~224 ns per 128x128x512 MM) but half the x-load DMA bytes.
  - Output stored fp16 (OUT_FP16), upcast to f32 on host: halves the
    out-store DMA. End-to-end max rel err 5.2e-4 vs the f32 reference
    (1.4e-4 if both flags are set back to f32r/False).
  - fp32 accumulation in PSUM throughout; context+bias folded into the
    PSUM->SBUF copy as a per-partition scalar add on the vector engine.
  - DMA loads/stores split across the SP and ACT hardware DGE queues.
PE is the bound: 320 matmuls x ~224 ns ~= 72 us/core; measured ~70-90
us/core sustained depending on terminal load.
"""

import sys

if "/opt/trn_rl_repo" not in sys.path:
    sys.path.insert(0, "/opt/trn_rl_repo")

import numpy as np

N, C_IN, L = 32, 256, 4096
C_OUT, K, GROUPS = 256, 9, 4
C_DIM, PAD = 128, 4
NCORES = 8
NB = N // NCORES          # batches per core
M = L // 2                # output phase positions (2048)
MPAD = (L + 2 * PAD) // 2  # padded phase length (2052)
MT = 512                  # phase cols per PSUM tile (one bank of fp32)
NMT = M // MT             # 4 L-tiles per (n, g)
NSHIFT = 5                # phase shifts (= ceil(K/2))
HALO = NSHIFT - 1

# Extra kwargs for run_bass_kernel_spmd (e.g. trace=True) set by a harness;
# the BassKernelResults lands in LAST_RESULT.
RUN_KWARGS: dict = {}
LAST_RESULT = None

_prog_cache: dict = {}

# Matmul input dtype for the conv path: "f32r" (safe, ~1.4e-4 rel err) or
# "fp16" (~2x less x-load DMA, ~2.8e-4 rel err).
CONV_DTYPE = "fp16"
# Store the output phase tensor as fp16 (halves out-DMA; host upcasts to
# f32; adds ~2.4e-4 output quantization).
OUT_FP16 = True
# One DMA per (n, g) row (525 KB load / 512 KB store) instead of 4 smaller
# tile DMAs: bigger lines, fewer descriptors.
BIG_TILES = True


def _build_program(repeats: int = 1, conv_dtype: str | None = None,
                   out_fp16: bool | None = None, big_tiles: bool | None = None):
    import concourse.bacc as bacc
    import concourse.mybir as mybir
    import concourse.tile as tile

    f32 = mybir.dt.float32
    f32r = {
        "f32r": mybir.dt.float32r,
        "fp16": mybir.dt.float16,
        "bf16": mybir.dt.bfloat16,
    }[conv_dtype or CONV_DTYPE]
    if out_fp16 is None:
        out_fp16 = OUT_FP16
    fout = mybir.dt.float16 if out_fp16 else f32

    nc = bacc.Bacc(None, target_bir_lowering=False, name="ctxconv1d")

    xq_d = nc.dram_tensor("xq", [NB, GROUPS, 128, MPAD], f32r, kind="ExternalInput")
    wq_d = nc.dram_tensor("wq", [128, GROUPS, NSHIFT, 128], f32r, kind="ExternalInput")
    cwT2_d = nc.dram_tensor("cwT2", [C_DIM, GROUPS, 128], f32, kind="ExternalInput")
    cT_d = nc.dram_tensor("cT", [C_DIM, NB], f32, kind="ExternalInput")
    biasT2_d = nc.dram_tensor("biasT2", [128, GROUPS], f32, kind="ExternalInput")
    outq_d = nc.dram_tensor("outq", [NB, GROUPS, 128, M], fout, kind="ExternalOutput")

    with tile.TileContext(nc) as tc:
        with (
            tc.tile_pool(name="consts", bufs=1) as consts,
            tc.tile_pool(name="xpool", bufs=8) as xpool,
            tc.tile_pool(name="opool", bufs=8) as opool,
            tc.tile_pool(name="ppool", bufs=4, space="PSUM") as ppool,
            tc.tile_pool(name="ctxp", bufs=1, space="PSUM") as ctxp,
        ):
            wq_sb = consts.tile([128, GROUPS, NSHIFT, 128], f32r)
            cwT2_sb = consts.tile([C_DIM, GROUPS, 128], f32)
            cT_sb = consts.tile([C_DIM, NB], f32)
            biasT2_sb = consts.tile([128, GROUPS], f32)
            ctx2_sb = consts.tile([128, GROUPS, NB], f32)

            nc.sync.dma_start(wq_sb[:], wq_d[:])
            nc.sync.dma_start(cwT2_sb[:], cwT2_d[:])
            nc.sync.dma_start(cT_sb[:], cT_d[:])
            nc.sync.dma_start(biasT2_sb[:], biasT2_d[:])

            # ctx2[(o,d), g, n] = sum_dim c_weight[g*64+o, dim] * c[n, dim] + bias
            # (columns duplicated across the two output parities d)
            for g in range(GROUPS):
                ctx_ps = ctxp.tile([128, NB], f32)
                nc.tensor.matmul(
                    ctx_ps[:], cwT2_sb[:, g, :], cT_sb[:], start=True, stop=True
                )
                nc.vector.tensor_scalar_add(
                    ctx2_sb[:, g, :], ctx_ps[:], biasT2_sb[:, g:g + 1]
                )

            use_big = BIG_TILES if big_tiles is None else big_tiles

            def body():
                idx = 0
                for n in range(NB):
                    for g in range(GROUPS):
                        ld = (nc.sync, nc.scalar)[idx % 2]
                        st = (nc.scalar, nc.sync)[idx % 2]
                        idx += 1
                        if use_big:
                            x_t = xpool.tile([128, MPAD], f32r)
                            ld.dma_start(x_t[:], xq_d[n, g, :, :])
                            o_t = opool.tile([128, M], fout)
                            for t in range(NMT):
                                ps = ppool.tile([128, MT], f32)
                                for s in range(NSHIFT):
                                    nc.tensor.matmul(
                                        ps[:],
                                        wq_sb[:, g, s, :],
                                        x_t[:, t * MT + s:t * MT + s + MT],
                                        start=(s == 0),
                                        stop=(s == NSHIFT - 1),
                                    )
                                nc.vector.tensor_scalar_add(
                                    o_t[:, t * MT:(t + 1) * MT], ps[:],
                                    ctx2_sb[:, g, n:n + 1],
                                )
                            st.dma_start(outq_d[n, g, :, :], o_t[:])
                        else:
                            for t in range(NMT):
                                ld = (nc.sync, nc.scalar)[idx % 2]
                                st = (nc.scalar, nc.sync)[idx % 2]
                                idx += 1
                                x_t = xpool.tile([128, MT + HALO], f32r)
                                ld.dma_start(
                                    x_t[:], xq_d[n, g, :, t * MT:t * MT + MT + HALO]
                                )
                                ps = ppool.tile([128, MT], f32)
                                for s in range(NSHIFT):
                                    nc.tensor.matmul(
                                        ps[:],
                                        wq_sb[:, g, s, :],
                                        x_t[:, s:s + MT],
                                        start=(s == 0),
                                        stop=(s == NSHIFT - 1),
                                    )
                                o_t = opool.tile([128, MT], fout)
                                nc.vector.tensor_scalar_add(
                                    o_t[:], ps[:], ctx2_sb[:, g, n:n + 1]
                                )
                                st.dma_start(
                                    outq_d[n, g, :, t * MT:(t + 1) * MT], o_t[:]
                                )

            if repeats == 1:
                body()
            else:
                # Big body (>256 insts/engine): arm back-edge prefetch so
                # repeat-loop timing isn't polluted by IRAM refetch stalls.
                with tc.For_i(
                    0, repeats, 1,
                    hint_engines=(
                        mybir.EngineType.PE,
                        mybir.EngineType.SP,
                        mybir.EngineType.Activation,
                        mybir.EngineType.DVE,
                        mybir.EngineType.Pool,
                    ),
                ):
                    body()

    nc.compile()
    return nc


def _get_program():
    if "nc" not in _prog_cache:
        _prog_cache["nc"] = _build_program()
    return _prog_cache["nc"]


def _conv_np_dtype(conv_dtype: str | None = None):
    import ml_dtypes

    return {
        "f32r": np.float32,
        "fp16": np.float16,
        "bf16": ml_dtypes.bfloat16,
    }[conv_dtype or CONV_DTYPE]


def _host_prep(x, c, weight, c_weight, bias, conv_dtype: str | None = None):
    # Phase-split padded x: xq[n, g, ph*64 + i, j] = xpad[n, g*64+i, 2j+ph]
    xp = np.zeros((N, C_IN, L + 2 * PAD), np.float32)
    xp[:, :, PAD:PAD + L] = x
    # (N, 4, 64, MPAD, 2) -> (N, 4, 2, 64, MPAD)
    xq = np.ascontiguousarray(
        xp.reshape(N, GROUPS, 64, MPAD, 2).transpose(0, 1, 4, 2, 3)
    ).reshape(N, GROUPS, 128, MPAD)

    # Polyphase stationary operands.
    wq = np.zeros((128, GROUPS, NSHIFT, 128), np.float32)
    for g in range(GROUPS):
        wg = weight[g * 64:(g + 1) * 64]          # (64 o, 64 i, K)
        for s in range(NSHIFT):
            wq[0:64, g, s, 0:64] = wg[:, :, 2 * s].T
            if 2 * s + 1 < K:
                wq[64:128, g, s, 0:64] = wg[:, :, 2 * s + 1].T
            if 2 * s - 1 >= 0:
                wq[0:64, g, s, 64:128] = wg[:, :, 2 * s - 1].T
            wq[64:128, g, s, 64:128] = wg[:, :, 2 * s].T

    # cwT2[d, g, 64*delta + o] = c_weight[g*64 + o, d]  (parity-duplicated)
    cwT2 = np.zeros((C_DIM, GROUPS, 128), np.float32)
    cw = c_weight.reshape(GROUPS, 64, C_DIM)
    for g in range(GROUPS):
        cwT2[:, g, 0:64] = cw[g].T
        cwT2[:, g, 64:128] = cw[g].T

    biasT2 = np.zeros((128, GROUPS), np.float32)
    b = bias.reshape(GROUPS, 64)
    biasT2[0:64] = b.T
    biasT2[64:128] = b.T

    cT = np.ascontiguousarray(c.T)  # (128, 32)

    npdt = _conv_np_dtype(conv_dtype)
    xq = xq.astype(npdt, copy=False)
    wq = wq.astype(npdt, copy=False)
    return xq, wq, cwT2, cT, biasT2


def kernel(x, c, weight, c_weight, bias):
    global LAST_RESULT
    from concourse.bass_utils import run_bass_kernel_spmd

    x = np.asarray(x, dtype=np.float32)
    c = np.asarray(c, dtype=np.float32)
    weight = np.asarray(weight, dtype=np.float32)
    c_weight = np.asarray(c_weight, dtype=np.float32)
    bias = np.asarray(bias, dtype=np.float32)

    xq, wq, cwT2, cT, biasT2 = _host_prep(x, c, weight, c_weight, bias)

    in_maps = []
    for i in range(NCORES):
        in_maps.append({
            "xq": np.ascontiguousarray(xq[i * NB:(i + 1) * NB]),
            "wq": wq,
            "cwT2": cwT2,
            "cT": np.ascontiguousarray(cT[:, i * NB:(i + 1) * NB]),
            "biasT2": biasT2,
        })

    nc = _get_program()
    res = run_bass_kernel_spmd(nc, in_maps, core_ids=list(range(NCORES)), **RUN_KWARGS)
    LAST_RESULT = res

    outq = np.concatenate([r["outq"] for r in res.results], axis=0)  # (N,4,128,M)
    # y[n, g*64+o, 2m+d] = outq[n, g, 64d+o, m]
    y = np.ascontiguousarray(
        outq.astype(np.float32, copy=False)
        .reshape(N, GROUPS, 2, 64, M)
        .transpose(0, 1, 3, 4, 2)
    ).reshape(N, C_OUT, L)
    return np.ascontiguousarray(y)

